# Initial kernel scaffold
#
"""Trainium2 Bass kernel for the DNC-style scatter_memory problem.

Contract: kernel(**inputs) takes the FULL unsharded numpy inputs (batch=64)
and returns the full outputs as the reference does:
    (memory_output [64,256], write_w [64,1024], updated [64,1024,1024])

Sharding: pure data-parallel over the batch axis across 8 NeuronCores
(8 batches per core); no cross-core communication.

Per-core algorithm highlights:
  * N=1024 vectors live in a [128,8] "column" layout (element n=c*128+p at
    [p,c]) so all elementwise work uses full partitions.
  * The reference's argsort-based allocation weighting is computed exactly
    (up to fp rounding, assuming distinct usage values) without sorting:
        alloc[i] = (1-u_i) * exp( sum_{j: u_j <= u_i} log u_j )
    The inner sum is evaluated with ACT Sign comparison tiles and small PE
    matmuls:  s_sign = D_sign^T @ log(u);  s_true = (s_sign + T + log u)/2.
  * The big memory-bound link update streams the 1024x1024 matrix in eight
    [128,1024] chunks, two fused scalar_tensor_tensor DVE passes per chunk:
        t   = (WJ + (1-w_i)) * L
        out = (PJ * w_i) + t
    The diagonal is zeroed afterwards by a strided DMA scatter into DRAM.
"""

import numpy as np

P = 128          # SBUF partitions
N = 1024         # MEMORY_SIZE
NCH = N // P     # 8 column chunks
A = 64           # ADDRESS_SIZE
R = 4            # READ_HEADS
IFW = 471        # interface width
OUTW = 256       # output size
BPC = 8          # batches per core
N_CORES = 8
EPS = 1e-6

_CACHE = {}


def _build_nc():
    import concourse.bass as bass
    import concourse.bacc as bacc
    import concourse.mybir as mybir
    from concourse import tile

    f32 = mybir.dt.float32
    Alu = mybir.AluOpType
    Act = mybir.ActivationFunctionType

    nc = bacc.Bacc(None, target_bir_lowering=False)

    interface = nc.dram_tensor("interface", [BPC, IFW], f32, kind="ExternalInput")
    memory = nc.dram_tensor("memory", [BPC, N, A], f32, kind="ExternalInput")
    read_weights = nc.dram_tensor("read_weights", [BPC, N, R], f32, kind="ExternalInput")
    write_weights = nc.dram_tensor("write_weights", [BPC, N], f32, kind="ExternalInput")
    usage_vec = nc.dram_tensor("usage_vec", [BPC, N], f32, kind="ExternalInput")
    precedence = nc.dram_tensor("precedence_weight", [BPC, N], f32, kind="ExternalInput")
    link = nc.dram_tensor("link_matrix", [BPC, N, N], f32, kind="ExternalInput")
    w_out = nc.dram_tensor("W_out", [IFW, OUTW], f32, kind="ExternalInput")
    b_out = nc.dram_tensor("b_out", [OUTW], f32, kind="ExternalInput")

    mem_out = nc.dram_tensor("memory_output", [BPC, OUTW], f32, kind="ExternalOutput")
    ww_out = nc.dram_tensor("write_w", [BPC, N], f32, kind="ExternalOutput")
    upd_out = nc.dram_tensor("updated", [BPC, N, N], f32, kind="ExternalOutput")

    def bc(ap, parts):
        # 0-stride partition broadcast view of a [1, ...] SBUF AP (DMA source)
        apl = list(ap.ap)
        assert apl[0][1] == 1, apl
        return bass.AP(ap.tensor, ap.offset, [[0, parts]] + [list(d) for d in apl[1:]])

    with tile.TileContext(nc) as tc:
        with (
            tc.tile_pool(name="const", bufs=1) as cpool,
            tc.tile_pool(name="small", bufs=3) as spool,
            tc.tile_pool(name="wide", bufs=2) as wpool,
            tc.tile_pool(name="dsign", bufs=3) as dpool,
            tc.tile_pool(name="lio", bufs=4) as lpool,
            tc.tile_pool(name="ps", bufs=2, space="PSUM") as pspool,
            tc.tile_pool(name="psm", bufs=1, space="PSUM") as psmpool,
        ):
            ones_col = cpool.tile([P, 1], f32)
            nc.vector.memset(ones_col[:], 1.0)
            zeros_row = cpool.tile([1, N], f32)
            nc.vector.memset(zeros_row[:], 0.0)

            # ---- dense output layer: mem_out = interface @ W_out + b_out ----
            ps_m = psmpool.tile([BPC, OUTW], f32)
            KCH = [(0, 128), (128, 128), (256, 128), (384, IFW - 384)]
            for ci, (k0, kd) in enumerate(KCH):
                ifT_c = cpool.tile([P, BPC], f32, name=f"ifT_{ci}")
                nc.sync.dma_start(ifT_c[:kd, :], interface[:, k0:k0 + kd].transpose([1, 0]))
                wo_c = cpool.tile([P, OUTW], f32, name=f"wo_{ci}")
                nc.sync.dma_start(wo_c[:kd, :], w_out[k0:k0 + kd, :])
                nc.tensor.matmul(ps_m[:], ifT_c[:kd, :], wo_c[:kd, :],
                                 start=(ci == 0), stop=(ci == len(KCH) - 1))
            b_row = cpool.tile([1, OUTW], f32)
            nc.sync.dma_start(b_row[:], b_out[:].unsqueeze(0))
            bb = cpool.tile([BPC, OUTW], f32)
            nc.sync.dma_start(bb[:], bc(b_row[:], BPC))
            mo = cpool.tile([BPC, OUTW], f32)
            nc.vector.tensor_add(mo[:], ps_m[:], bb[:])
            nc.sync.dma_start(mem_out[:], mo[:])

            # ---- per-batch pipeline ----
            for b in range(BPC):
                # interface row and derived scalars
                if_row = spool.tile([1, IFW], f32)
                nc.sync.dma_start(if_row[:], interface[b:b + 1, :])

                sc_row = spool.tile([1, 8], f32)
                fg_t = spool.tile([1, R], f32)
                nc.scalar.activation(fg_t[:], if_row[0:1, 192:196], Act.Sigmoid)
                # sc[0:4] = -free_gate
                nc.vector.tensor_scalar_mul(sc_row[0:1, 0:4], fg_t[:], -1.0)
                sp_t = spool.tile([1, 1], f32)
                nc.scalar.activation(sp_t[:], if_row[0:1, 200:201], Act.Softplus)
                # sc[4] = 1 + softplus = write_strength
                nc.vector.tensor_scalar_add(sc_row[0:1, 4:5], sp_t[:], 1.0)
                g_t = spool.tile([1, 2], f32)
                nc.scalar.activation(g_t[:], if_row[0:1, 201:203], Act.Sigmoid)
                ag1_t = spool.tile([1, 1], f32)
                # ag1 = 1 - alloc_gate
                nc.vector.tensor_scalar(ag1_t[:], g_t[0:1, 1:2], -1.0, 1.0,
                                        op0=Alu.mult, op1=Alu.add)
                # sc[5] = write_gate * (1 - alloc_gate)
                nc.vector.tensor_tensor(sc_row[0:1, 5:6], g_t[0:1, 0:1], ag1_t[:], op=Alu.mult)
                # sc[6] = -(write_gate * alloc_gate)
                nc.vector.scalar_tensor_tensor(sc_row[0:1, 6:7], g_t[0:1, 1:2], -1.0,
                                               g_t[0:1, 0:1], op0=Alu.mult, op1=Alu.mult)
                ksq_t = spool.tile([1, A], f32)
                ksa_t = spool.tile([1, 1], f32)
                nc.scalar.activation(ksq_t[:], if_row[0:1, 0:A], Act.Square,
                                     accum_out=ksa_t[:])
                # sc[7] = |write_key|
                nc.scalar.activation(sc_row[0:1, 7:8], ksa_t[:], Act.Sqrt)
                sc_b = spool.tile([P, 8], f32)
                nc.sync.dma_start(sc_b[:], bc(sc_row[:], P))

                # retention / usage (column layout)
                rw_col = spool.tile([P, NCH, R], f32)
                nc.sync.dma_start(rw_col[:], read_weights[b].rearrange("(c p) r -> p c r", p=P))
                ret = spool.tile([P, NCH], f32)
                tr = spool.tile([P, NCH], f32)
                nc.vector.tensor_scalar(ret[:], rw_col[:, :, 0], sc_b[:, 0:1], 1.0,
                                        op0=Alu.mult, op1=Alu.add)
                for r in range(1, R):
                    nc.vector.tensor_scalar(tr[:], rw_col[:, :, r], sc_b[:, r:r + 1], 1.0,
                                            op0=Alu.mult, op1=Alu.add)
                    nc.vector.tensor_tensor(ret[:], ret[:], tr[:], op=Alu.mult)

                u_col = spool.tile([P, NCH], f32)
                nc.sync.dma_start(u_col[:], usage_vec[b].rearrange("(c p) -> p c", p=P))
                wv_col = spool.tile([P, NCH], f32)
                nc.sync.dma_start(wv_col[:], write_weights[b].rearrange("(c p) -> p c", p=P))
                t1 = spool.tile([P, NCH], f32)
                t2 = spool.tile([P, NCH], f32)
                nc.vector.tensor_add(t1[:], u_col[:], wv_col[:])
                nc.vector.tensor_mul(t2[:], u_col[:], wv_col[:])
                nc.vector.tensor_sub(t1[:], t1[:], t2[:])
                usage = spool.tile([P, NCH], f32)
                nc.vector.tensor_tensor(usage[:], t1[:], ret[:], op=Alu.mult)
                neg_u = spool.tile([P, NCH], f32)
                nc.vector.tensor_scalar_mul(neg_u[:], usage[:], -1.0)
                logu = spool.tile([P, NCH], f32)
                lacc = spool.tile([P, 1], f32)
                nc.scalar.activation(logu[:], usage[:], Act.Ln, accum_out=lacc[:])

                # usage broadcast row for the comparison tiles
                u_row = spool.tile([1, N], f32)
                nc.sync.dma_start(
                    bass.AP(u_row.tensor, u_row.offset,
                            [list(u_row.ap[0]), [1, P], [P, NCH]]),
                    usage[:])
                u_b = wpool.tile([P, N], f32, name="u_b")
                nc.sync.dma_start(u_b[:], bc(u_row[:], P))

                # allocation-sort surrogate:
                # D_k[p,i] = sign(u_i - u_{k*128+p}); s_sign = sum_k D_k^T @ logu_k
                ps_s = pspool.tile([P, NCH], f32)
                for kc in range(NCH):
                    d_k = dpool.tile([P, N], f32, name="d_k")
                    nc.scalar.activation(d_k[:], u_b[:], Act.Sign,
                                         bias=neg_u[:, kc:kc + 1])
                    for mc in range(NCH):
                        nc.tensor.matmul(ps_s[:, mc:mc + 1],
                                         d_k[:, mc * P:(mc + 1) * P],
                                         logu[:, kc:kc + 1],
                                         start=(kc == 0), stop=(kc == NCH - 1))
                s_mid = spool.tile([P, NCH], f32)
                nc.vector.tensor_add(s_mid[:], ps_s[:], logu[:])

                # content addressing
                mem_col = spool.tile([P, NCH, A], f32)
                nc.sync.dma_start(mem_col[:], memory[b].rearrange("(c p) a -> p c a", p=P))
                kb8 = spool.tile([P, NCH, A], f32)
                nc.sync.dma_start(
                    kb8[:],
                    bass.AP(if_row.tensor, if_row.offset, [[0, P], [0, NCH], [1, A]]))
                dm = spool.tile([P, NCH, A], f32)
                nc.vector.tensor_mul(dm[:], mem_col[:], kb8[:])
                dot = spool.tile([P, NCH], f32)
                nc.vector.reduce_sum(dot[:], dm[:], axis=mybir.AxisListType.X)
                msq = spool.tile([P, NCH, A], f32)
                nc.scalar.activation(msq[:], mem_col[:], Act.Square)
                nsq = spool.tile([P, NCH], f32)
                nc.vector.reduce_sum(nsq[:], msq[:], axis=mybir.AxisListType.X)
                mn = spool.tile([P, NCH], f32)
                nc.scalar.activation(mn[:], nsq[:], Act.Sqrt)
                den = spool.tile([P, NCH], f32)
                nc.vector.tensor_scalar(den[:], mn[:], sc_b[:, 7:8], EPS,
                                        op0=Alu.mult, op1=Alu.add)
                rec = spool.tile([P, NCH], f32)
                nc.vector.reciprocal(rec[:], den[:])
                sim_s = spool.tile([P, NCH], f32)
                nc.vector.scalar_tensor_tensor(sim_s[:], dot[:], sc_b[:, 4:5], rec[:],
                                               op0=Alu.mult, op1=Alu.mult)
                e_t = spool.tile([P, NCH], f32)
                eacc = spool.tile([P, 1], f32)
                nc.scalar.activation(e_t[:], sim_s[:], Act.Exp, accum_out=eacc[:])

                # cross-partition totals via PE
                ps_t = pspool.tile([1, 2], f32)
                nc.tensor.matmul(ps_t[:, 0:1], eacc[:], ones_col[:], start=True, stop=True)
                nc.tensor.matmul(ps_t[:, 1:2], lacc[:], ones_col[:], start=True, stop=True)
                sc2_row = spool.tile([1, 2], f32)
                nc.vector.reciprocal(sc2_row[0:1, 0:1], ps_t[:, 0:1])
                nc.vector.tensor_scalar_mul(sc2_row[0:1, 1:2], ps_t[:, 1:2], 0.5)
                sc2_b = spool.tile([P, 2], f32)
                nc.sync.dma_start(sc2_b[:], bc(sc2_row[:], P))

                # write weighting
                wa = spool.tile([P, NCH], f32)
                nc.vector.tensor_scalar_mul(wa[:], e_t[:], sc2_b[:, 0:1])
                palloc = spool.tile([P, NCH], f32)
                nc.scalar.activation(palloc[:], s_mid[:], Act.Exp,
                                     bias=sc2_b[:, 1:2], scale=0.5)
                aneg = spool.tile([P, NCH], f32)
                nc.vector.scalar_tensor_tensor(aneg[:], usage[:], 1.0, palloc[:],
                                               op0=Alu.subtract, op1=Alu.mult)
                ta = spool.tile([P, NCH], f32)
                nc.vector.tensor_scalar_mul(ta[:], aneg[:], sc_b[:, 6:7])
                ww = spool.tile([P, NCH], f32)
                nc.vector.scalar_tensor_tensor(ww[:], wa[:], sc_b[:, 5:6], ta[:],
                                               op0=Alu.mult, op1=Alu.add)
                acol = spool.tile([P, NCH], f32)
                nc.vector.tensor_scalar(acol[:], ww[:], -1.0, 1.0,
                                        op0=Alu.mult, op1=Alu.add)

                # write_w to DRAM (column -> natural order scatter)
                nc.sync.dma_start(
                    bass.AP(ww_out.tensor, b * N, [[1, P], [P, NCH]]), ww[:])

                # broadcasts for the link update
                ww_row = spool.tile([1, N], f32)
                nc.sync.dma_start(
                    bass.AP(ww_row.tensor, ww_row.offset,
                            [list(ww_row.ap[0]), [1, P], [P, NCH]]),
                    ww[:])
                wj = wpool.tile([P, N], f32, name="wj")
                nc.sync.dma_start(wj[:], bc(ww_row[:], P))
                p_row = spool.tile([1, N], f32)
                nc.sync.dma_start(p_row[:], precedence[b:b + 1, :])
                pj = wpool.tile([P, N], f32, name="pj")
                nc.sync.dma_start(pj[:], bc(p_row[:], P))

                # link update, 8 chunks of [128, 1024]
                lch = link[b].rearrange("(c p) j -> c p j", p=P)
                uch = upd_out[b].rearrange("(c p) j -> c p j", p=P)
                for c in range(NCH):
                    l_c = lpool.tile([P, N], f32, name="l_c")
                    nc.sync.dma_start(l_c[:], lch[c])
                    o_c = lpool.tile([P, N], f32, name="o_c")
                    nc.vector.scalar_tensor_tensor(o_c[:], wj[:], acol[:, c:c + 1],
                                                   l_c[:], op0=Alu.add, op1=Alu.mult)
                    nc.vector.scalar_tensor_tensor(o_c[:], pj[:], ww[:, c:c + 1],
                                                   o_c[:], op0=Alu.mult, op1=Alu.add)
                    nc.sync.dma_start(uch[c], o_c[:])
                # zero the diagonal
                nc.sync.dma_start(
                    bass.AP(upd_out.tensor, b * N * N, [[N + 1, N]]),
                    zeros_row[:])

    return nc


def _get_nc():
    if "nc" not in _CACHE:
        _CACHE["nc"] = _build_nc()
        _CACHE["nc"].compile()
    return _CACHE["nc"]


def kernel(interface, memory, read_weights, write_weights, usage_vec,
           precedence_weight, link_matrix, W_out, b_out):
    from concourse.bass_utils import run_bass_kernel_spmd

    nc = _get_nc()
    f = np.float32
    full = {
        "interface": np.ascontiguousarray(interface, dtype=f),
        "memory": np.ascontiguousarray(memory, dtype=f),
        "read_weights": np.ascontiguousarray(read_weights, dtype=f),
        "write_weights": np.ascontiguousarray(write_weights, dtype=f),
        "usage_vec": np.ascontiguousarray(usage_vec, dtype=f),
        "precedence_weight": np.ascontiguousarray(precedence_weight, dtype=f),
        "link_matrix": np.ascontiguousarray(link_matrix, dtype=f),
    }
    shared = {
        "W_out": np.ascontiguousarray(W_out, dtype=f),
        "b_out": np.ascontiguousarray(b_out, dtype=f),
    }
    in_maps = []
    for c in range(N_CORES):
        m = {k: v[c * BPC:(c + 1) * BPC] for k, v in full.items()}
        m.update(shared)
        in_maps.append(m)

    res = run_bass_kernel_spmd(nc, in_maps, core_ids=list(range(N_CORES)))
    mo = np.concatenate([res.results[c]["memory_output"] for c in range(N_CORES)], axis=0)
    ww = np.concatenate([res.results[c]["write_w"] for c in range(N_CORES)], axis=0)
    upd = np.concatenate([res.results[c]["updated"] for c in range(N_CORES)], axis=0)
    return mo, ww, upd


# revision 8
# speedup vs baseline: 1.1719x; 1.1719x over previous
"""Trainium2 Bass kernel for the DNC-style scatter_memory problem.

Contract: kernel(**inputs) takes the FULL unsharded numpy inputs (batch=64)
and returns the full outputs as the reference does:
    (memory_output [64,256], write_w [64,1024], updated [64,1024,1024])

Sharding: pure data-parallel over the batch axis across 8 NeuronCores
(8 batches per core); no cross-core communication.

Per-core algorithm highlights:
  * N=1024 vectors live in a [128,8] "column" layout (element n=c*128+p at
    [p,c]) so all elementwise work uses full partitions.
  * The reference's argsort-based allocation weighting is computed exactly
    (up to fp rounding, assuming distinct usage values) without sorting:
        alloc[i] = (1-u_i) * exp( sum_{j: u_j <= u_i} log u_j )
    The inner sum is evaluated with ACT Sign comparison tiles and small PE
    matmuls:  s_sign = D_sign^T @ log(u);  s_true = (s_sign + T + log u)/2.
  * The big memory-bound link update streams the 1024x1024 matrix in eight
    [128,1024] chunks, two fused scalar_tensor_tensor DVE passes per chunk:
        t   = (WJ + (1-w_i)) * L
        out = (PJ * w_i) + t
    The diagonal is zeroed afterwards by a strided DMA scatter into DRAM.
"""

import numpy as np

P = 128          # SBUF partitions
N = 1024         # MEMORY_SIZE
NCH = N // P     # 8 column chunks
A = 64           # ADDRESS_SIZE
R = 4            # READ_HEADS
IFW = 471        # interface width
OUTW = 256       # output size
BPC = 8          # batches per core
N_CORES = 8
EPS = 1e-6

_CACHE = {}


def _build_nc(repeat: int = 1):
    import concourse.bass as bass
    import concourse.bacc as bacc
    import concourse.mybir as mybir
    from concourse import tile

    f32 = mybir.dt.float32
    Alu = mybir.AluOpType
    Act = mybir.ActivationFunctionType

    nc = bacc.Bacc(None, target_bir_lowering=False)

    interface = nc.dram_tensor("interface", [BPC, IFW], f32, kind="ExternalInput")
    memory = nc.dram_tensor("memory", [BPC, N, A], f32, kind="ExternalInput")
    read_weights = nc.dram_tensor("read_weights", [BPC, N, R], f32, kind="ExternalInput")
    write_weights = nc.dram_tensor("write_weights", [BPC, N], f32, kind="ExternalInput")
    usage_vec = nc.dram_tensor("usage_vec", [BPC, N], f32, kind="ExternalInput")
    precedence = nc.dram_tensor("precedence_weight", [BPC, N], f32, kind="ExternalInput")
    link = nc.dram_tensor("link_matrix", [BPC, N, N], f32, kind="ExternalInput")
    w_out = nc.dram_tensor("W_out", [IFW, OUTW], f32, kind="ExternalInput")
    b_out = nc.dram_tensor("b_out", [OUTW], f32, kind="ExternalInput")

    mem_out = nc.dram_tensor("memory_output", [BPC, OUTW], f32, kind="ExternalOutput")
    ww_out = nc.dram_tensor("write_w", [BPC, N], f32, kind="ExternalOutput")
    upd_out = nc.dram_tensor("updated", [BPC, N, N], f32, kind="ExternalOutput")

    def bc(ap, parts):
        # broadcast view of a [1, ...] SBUF AP for DMA: keep the count-1
        # partition dim, insert a 0-step free dim that repeats the row
        apl = [list(d) for d in ap.ap]
        assert apl[0][1] == 1, apl
        return bass.AP(ap.tensor, ap.offset, [apl[0], [0, parts]] + apl[1:])

    with tile.TileContext(nc) as tc:
        with (
            tc.tile_pool(name="const", bufs=1) as cpool,
            tc.tile_pool(name="small", bufs=3) as spool,
            tc.tile_pool(name="wide", bufs=2) as wpool,
            tc.tile_pool(name="dsign", bufs=3) as dpool,
            tc.tile_pool(name="lio", bufs=4) as lpool,
            tc.tile_pool(name="dram", bufs=1, space="DRAM") as drpool,
            tc.tile_pool(name="ps", bufs=2, space="PSUM") as pspool,
            tc.tile_pool(name="psm", bufs=1, space="PSUM") as psmpool,
        ):
            ones_col = cpool.tile([P, 1], f32)
            nc.vector.memset(ones_col[:], 1.0)
            zeros_row = cpool.tile([1, N], f32)
            nc.vector.memset(zeros_row[:], 0.0)
            u_scr = drpool.tile([BPC, N], f32)
            w_scr = drpool.tile([BPC, N], f32)

            # ---- dense output layer: mem_out = interface @ W_out + b_out ----
            ps_m = psmpool.tile([BPC, OUTW], f32)
            KCH = [(0, 128), (128, 128), (256, 128), (384, IFW - 384)]
            for ci, (k0, kd) in enumerate(KCH):
                ifT_c = cpool.tile([P, BPC], f32, name=f"ifT_{ci}")
                nc.sync.dma_start(ifT_c[:kd, :], interface[:, k0:k0 + kd].transpose([1, 0]))
                wo_c = cpool.tile([P, OUTW], f32, name=f"wo_{ci}")
                nc.sync.dma_start(wo_c[:kd, :], w_out[k0:k0 + kd, :])
                nc.tensor.matmul(ps_m[:], ifT_c[:kd, :], wo_c[:kd, :],
                                 start=(ci == 0), stop=(ci == len(KCH) - 1))
            b_row = cpool.tile([1, OUTW], f32)
            nc.sync.dma_start(b_row[:], b_out[:].unsqueeze(0))
            bb = cpool.tile([BPC, OUTW], f32)
            nc.sync.dma_start(bb[:], bc(b_row[:], BPC))
            mo = cpool.tile([BPC, OUTW], f32)
            nc.vector.tensor_add(mo[:], ps_m[:], bb[:])
            nc.sync.dma_start(mem_out[:], mo[:])

            # ---- per-batch pipeline (repeat>1 only for timing builds) ----
            for b in [b for _ in range(repeat) for b in range(BPC)]:
                # interface row and derived scalars
                if_row = spool.tile([1, IFW], f32)
                nc.sync.dma_start(if_row[:], interface[b:b + 1, :])

                # Only Exp/Ln/Sign/Square ACT functions are used anywhere in
                # this kernel, so the activation table is loaded exactly once.
                sc_row = spool.tile([1, 8], f32)
                # sigmoid of [free_gate(4), write_gate, alloc_gate] via exp
                sg_in = spool.tile([1, 6], f32)
                nc.vector.tensor_copy(sg_in[0:1, 0:4], if_row[0:1, 192:196])
                nc.vector.tensor_copy(sg_in[0:1, 4:6], if_row[0:1, 201:203])
                sg_e = spool.tile([1, 6], f32)
                nc.scalar.activation(sg_e[:], sg_in[:], Act.Exp, scale=-1.0)
                nc.vector.tensor_scalar_add(sg_e[:], sg_e[:], 1.0)
                sg_t = spool.tile([1, 6], f32)
                nc.vector.reciprocal(sg_t[:], sg_e[:])
                # sc[0:4] = -free_gate
                nc.vector.tensor_scalar_mul(sc_row[0:1, 0:4], sg_t[0:1, 0:4], -1.0)
                # softplus(x) = ln(1 + exp(x))
                sp_e = spool.tile([1, 1], f32)
                nc.scalar.activation(sp_e[:], if_row[0:1, 200:201], Act.Exp)
                nc.vector.tensor_scalar_add(sp_e[:], sp_e[:], 1.0)
                sp_t = spool.tile([1, 1], f32)
                nc.scalar.activation(sp_t[:], sp_e[:], Act.Ln)
                # 1/|write_key| = exp(-0.5 ln(sum k^2))
                ksq_t = spool.tile([1, A], f32)
                ksa_t = spool.tile([1, 1], f32)
                nc.scalar.activation(ksq_t[:], if_row[0:1, 0:A], Act.Square,
                                     accum_out=ksa_t[:])
                kln_t = spool.tile([1, 1], f32)
                nc.scalar.activation(kln_t[:], ksa_t[:], Act.Ln)
                ikn_t = spool.tile([1, 1], f32)
                nc.scalar.activation(ikn_t[:], kln_t[:], Act.Exp, scale=-0.5)
                # sc[4] = write_strength / |write_key| = (1 + softplus) * ikn
                st_t = spool.tile([1, 1], f32)
                nc.vector.tensor_scalar_add(st_t[:], sp_t[:], 1.0)
                nc.vector.tensor_tensor(sc_row[0:1, 4:5], st_t[:], ikn_t[:], op=Alu.mult)
                ag1_t = spool.tile([1, 1], f32)
                # ag1 = 1 - alloc_gate
                nc.vector.tensor_scalar(ag1_t[:], sg_t[0:1, 5:6], -1.0, 1.0,
                                        op0=Alu.mult, op1=Alu.add)
                # sc[5] = write_gate * (1 - alloc_gate)
                nc.vector.tensor_tensor(sc_row[0:1, 5:6], sg_t[0:1, 4:5], ag1_t[:], op=Alu.mult)
                # sc[6] = -(write_gate * alloc_gate)
                nc.vector.scalar_tensor_tensor(sc_row[0:1, 6:7], sg_t[0:1, 5:6], -1.0,
                                               sg_t[0:1, 4:5], op0=Alu.mult, op1=Alu.mult)
                # sc[7] unused (kept initialized)
                nc.vector.tensor_copy(sc_row[0:1, 7:8], ikn_t[:])
                sc_b = spool.tile([P, 8], f32)
                nc.sync.dma_start(sc_b[:], bc(sc_row[:], P))

                # retention / usage (column layout)
                rw_col = spool.tile([P, NCH, R], f32)
                nc.sync.dma_start(rw_col[:], read_weights[b].rearrange("(c p) r -> p c r", p=P))
                ret = spool.tile([P, NCH], f32)
                tr = spool.tile([P, NCH], f32)
                nc.vector.tensor_scalar(ret[:], rw_col[:, :, 0], sc_b[:, 0:1], 1.0,
                                        op0=Alu.mult, op1=Alu.add)
                for r in range(1, R):
                    nc.vector.tensor_scalar(tr[:], rw_col[:, :, r], sc_b[:, r:r + 1], 1.0,
                                            op0=Alu.mult, op1=Alu.add)
                    nc.vector.tensor_tensor(ret[:], ret[:], tr[:], op=Alu.mult)

                u_col = spool.tile([P, NCH], f32)
                nc.sync.dma_start(u_col[:], usage_vec[b].rearrange("(c p) -> p c", p=P))
                wv_col = spool.tile([P, NCH], f32)
                nc.sync.dma_start(wv_col[:], write_weights[b].rearrange("(c p) -> p c", p=P))
                t1 = spool.tile([P, NCH], f32)
                t2 = spool.tile([P, NCH], f32)
                nc.vector.tensor_add(t1[:], u_col[:], wv_col[:])
                nc.vector.tensor_mul(t2[:], u_col[:], wv_col[:])
                nc.vector.tensor_sub(t1[:], t1[:], t2[:])
                usage = spool.tile([P, NCH], f32)
                nc.vector.tensor_tensor(usage[:], t1[:], ret[:], op=Alu.mult)
                neg_u = spool.tile([P, NCH], f32)
                nc.vector.tensor_scalar_mul(neg_u[:], usage[:], -1.0)
                logu = spool.tile([P, NCH], f32)
                lacc = spool.tile([P, 1], f32)
                nc.scalar.activation(logu[:], usage[:], Act.Ln, accum_out=lacc[:])

                # usage broadcast row for the comparison tiles (via DRAM bounce)
                nc.sync.dma_start(
                    bass.AP(u_scr.tensor, u_scr.offset + b * N, [[1, P], [P, NCH]]),
                    usage[:])
                u_row = spool.tile([1, N], f32)
                nc.sync.dma_start(u_row[:], u_scr[b:b + 1, :])
                u_b = wpool.tile([P, N], f32, name="u_b")
                nc.sync.dma_start(u_b[:], bc(u_row[:], P))

                # allocation-sort surrogate:
                # D_k[p,i] = sign(u_i - u_{k*128+p}); s_sign = sum_k D_k^T @ logu_k
                d_ks = []
                for kc in range(NCH):
                    d_k = dpool.tile([P, N], f32, name=f"d_k{kc}", tag="d_k", bufs=9)
                    nc.scalar.activation(d_k[:], u_b[:], Act.Sign,
                                         bias=neg_u[:, kc:kc + 1])
                    d_ks.append(d_k)
                s_mid = spool.tile([P, NCH], f32)
                for mc in range(NCH):
                    # one accumulation group per single-bank psum tile
                    ps_c = pspool.tile([P, 1], f32, name="ps_c", tag="ps_c", bufs=2)
                    for kc in range(NCH):
                        nc.tensor.matmul(ps_c[:],
                                         d_ks[kc][:, mc * P:(mc + 1) * P],
                                         logu[:, kc:kc + 1],
                                         start=(kc == 0), stop=(kc == NCH - 1))
                    nc.vector.tensor_add(s_mid[:, mc:mc + 1], ps_c[:], logu[:, mc:mc + 1])

                # content addressing
                mem_col = spool.tile([P, NCH, A], f32)
                nc.sync.dma_start(mem_col[:], memory[b].rearrange("(c p) a -> p c a", p=P))
                k8_row = spool.tile([1, NCH * A], f32)
                nc.sync.dma_start(
                    k8_row[:],
                    bass.AP(if_row.tensor, if_row.offset, [list(if_row.ap[0]), [0, NCH], [1, A]]))
                kb8 = spool.tile([P, NCH, A], f32)
                nc.sync.dma_start(kb8[:], bc(k8_row[:], P))
                dm = spool.tile([P, NCH, A], f32)
                nc.vector.tensor_mul(dm[:], mem_col[:], kb8[:])
                dot = spool.tile([P, NCH], f32)
                nc.vector.reduce_sum(dot[:], dm[:], axis=mybir.AxisListType.X)
                msq = spool.tile([P, NCH, A], f32)
                nc.scalar.activation(msq[:], mem_col[:], Act.Square)
                nsq = spool.tile([P, NCH], f32)
                nc.vector.reduce_sum(nsq[:], msq[:], axis=mybir.AxisListType.X)
                # 1/mem_norm = exp(-0.5 ln(nsq)); eps in the reference denom is
                # negligible (denominators are O(64))
                lnn = spool.tile([P, NCH], f32)
                nc.scalar.activation(lnn[:], nsq[:], Act.Ln)
                rsq = spool.tile([P, NCH], f32)
                nc.scalar.activation(rsq[:], lnn[:], Act.Exp, scale=-0.5)
                sim_s = spool.tile([P, NCH], f32)
                nc.vector.scalar_tensor_tensor(sim_s[:], dot[:], sc_b[:, 4:5], rsq[:],
                                               op0=Alu.mult, op1=Alu.mult)
                e_t = spool.tile([P, NCH], f32)
                eacc = spool.tile([P, 1], f32)
                nc.scalar.activation(e_t[:], sim_s[:], Act.Exp, accum_out=eacc[:])

                # cross-partition totals via PE (separate single-bank tiles)
                ps_e = pspool.tile([1, 1], f32, tag="ps_e", bufs=2)
                ps_T = pspool.tile([1, 1], f32, tag="ps_T", bufs=2)
                nc.tensor.matmul(ps_e[:], eacc[:], ones_col[:], start=True, stop=True)
                nc.tensor.matmul(ps_T[:], lacc[:], ones_col[:], start=True, stop=True)
                sc2_row = spool.tile([1, 2], f32)
                nc.vector.reciprocal(sc2_row[0:1, 0:1], ps_e[:])
                nc.vector.tensor_scalar_mul(sc2_row[0:1, 1:2], ps_T[:], 0.5)
                sc2_b = spool.tile([P, 2], f32)
                nc.sync.dma_start(sc2_b[:], bc(sc2_row[:], P))

                # write weighting
                wa = spool.tile([P, NCH], f32)
                nc.vector.tensor_scalar_mul(wa[:], e_t[:], sc2_b[:, 0:1])
                palloc = spool.tile([P, NCH], f32)
                nc.scalar.activation(palloc[:], s_mid[:], Act.Exp,
                                     bias=sc2_b[:, 1:2], scale=0.5)
                aneg = spool.tile([P, NCH], f32)
                nc.vector.scalar_tensor_tensor(aneg[:], usage[:], 1.0, palloc[:],
                                               op0=Alu.subtract, op1=Alu.mult)
                ta = spool.tile([P, NCH], f32)
                nc.vector.tensor_scalar_mul(ta[:], aneg[:], sc_b[:, 6:7])
                ww = spool.tile([P, NCH], f32)
                nc.vector.scalar_tensor_tensor(ww[:], wa[:], sc_b[:, 5:6], ta[:],
                                               op0=Alu.mult, op1=Alu.add)
                acol = spool.tile([P, NCH], f32)
                nc.vector.tensor_scalar(acol[:], ww[:], -1.0, 1.0,
                                        op0=Alu.mult, op1=Alu.add)

                # write_w: column -> natural order via DRAM bounce
                nc.sync.dma_start(
                    bass.AP(w_scr.tensor, w_scr.offset + b * N, [[1, P], [P, NCH]]),
                    ww[:])
                ww_row = spool.tile([1, N], f32)
                nc.sync.dma_start(ww_row[:], w_scr[b:b + 1, :])
                nc.sync.dma_start(ww_out[b:b + 1, :], ww_row[:])
                wj = wpool.tile([P, N], f32, name="wj")
                nc.sync.dma_start(wj[:], bc(ww_row[:], P))
                p_row = spool.tile([1, N], f32)
                nc.sync.dma_start(p_row[:], precedence[b:b + 1, :])
                pj = wpool.tile([P, N], f32, name="pj")
                nc.sync.dma_start(pj[:], bc(p_row[:], P))

                # link update, 8 chunks of [128, 1024]
                lch = link[b].rearrange("(c p) j -> c p j", p=P)
                uch = upd_out[b].rearrange("(c p) j -> c p j", p=P)
                for c in range(NCH):
                    l_c = lpool.tile([P, N], f32, name="l_c")
                    nc.sync.dma_start(l_c[:], lch[c])
                    o_c = lpool.tile([P, N], f32, name="o_c")
                    nc.vector.scalar_tensor_tensor(o_c[:], wj[:], acol[:, c:c + 1],
                                                   l_c[:], op0=Alu.add, op1=Alu.mult)
                    nc.vector.scalar_tensor_tensor(o_c[:], pj[:], ww[:, c:c + 1],
                                                   o_c[:], op0=Alu.mult, op1=Alu.add)
                    nc.sync.dma_start(uch[c], o_c[:])
                # zero the diagonal
                nc.sync.dma_start(
                    bass.AP(upd_out, b * N * N, [[N + 1, N]]),
                    zeros_row[:])

    return nc


def _get_nc(repeat: int = 1):
    key = ("nc", repeat)
    if key not in _CACHE:
        _CACHE[key] = _build_nc(repeat)
        _CACHE[key].compile()
    return _CACHE[key]


def kernel(interface, memory, read_weights, write_weights, usage_vec,
           precedence_weight, link_matrix, W_out, b_out):
    from concourse.bass_utils import run_bass_kernel_spmd

    nc = _get_nc()
    f = np.float32
    full = {
        "interface": np.ascontiguousarray(interface, dtype=f),
        "memory": np.ascontiguousarray(memory, dtype=f),
        "read_weights": np.ascontiguousarray(read_weights, dtype=f),
        "write_weights": np.ascontiguousarray(write_weights, dtype=f),
        "usage_vec": np.ascontiguousarray(usage_vec, dtype=f),
        "precedence_weight": np.ascontiguousarray(precedence_weight, dtype=f),
        "link_matrix": np.ascontiguousarray(link_matrix, dtype=f),
    }
    shared = {
        "W_out": np.ascontiguousarray(W_out, dtype=f),
        "b_out": np.ascontiguousarray(b_out, dtype=f),
    }
    in_maps = []
    for c in range(N_CORES):
        m = {k: v[c * BPC:(c + 1) * BPC] for k, v in full.items()}
        m.update(shared)
        in_maps.append(m)

    res = run_bass_kernel_spmd(nc, in_maps, core_ids=list(range(N_CORES)))
    mo = np.concatenate([res.results[c]["memory_output"] for c in range(N_CORES)], axis=0)
    ww = np.concatenate([res.results[c]["write_w"] for c in range(N_CORES)], axis=0)
    upd = np.concatenate([res.results[c]["updated"] for c in range(N_CORES)], axis=0)
    return mo, ww, upd
